# revision 1
# baseline (speedup 1.0000x reference)
"""Multi-head causal attention (B=4, S=2048, H=16, Dh=64, Dm=1024) on 8
Trainium2 NeuronCores.

Sharding: core c handles batch b = c//2 and heads [8*(c%2), 8*(c%2)+8).
Each core computes its 8 heads' full attention + O-projection partial sum;
the host adds the two half-head partials per batch plus O_b.

Device layout (per core, all matmuls in fp32r = full-rate single-pass fp32):
  - host passes xT = x[b].T so the contraction dim (d_model) is the SBUF
    partition dim everywhere; no on-device transposes are ever needed.
  - qT/kT computed per head-PAIR in [2*64=128 rows (d of 2 heads), seq]
    layout; logitsT[kp, qp] is computed with 2 concurrent row-tiled K=64
    matmuls (head A in PE rows 0-63, head B in rows 64-127), softmax'd
    un-normalized via ACT exp (scale=1/8 folded in), causal-masked by a
    precomputed staircase multiply, then consumed directly as the moving
    operand of col-tiled S@V matmuls (head A -> PSUM rows 0-63, head B ->
    rows 64-127).  Denominators come from col-tiled M=1 ones-matmuls, are
    reciprocal'd on DVE and broadcast across partitions with K=1
    outer-product matmuls, so the softmax division is one DVE multiply.
"""

import os
import sys

sys.path.insert(0, "/opt/trn_rl_repo")

import numpy as np

B, S, DM, H, DH = 4, 2048, 1024, 16, 64
HPC = 8          # heads per core
NPAIR = HPC // 2
PB = 512         # qp block width
KT = 128         # kp tile width
NQP = S // PB    # 4 qp blocks
MT = DM // 128   # 8 m-tiles

_cache = {}


def _split_multi_waits(nc, mybir):
    # This container's walrus rejects >1 sync wait per instruction
    # ("Too many sync wait commands").  Move extra waits onto same-engine
    # NoOps right before the instruction; per-engine program order makes
    # this equivalent.
    ctr = 0
    for fn in nc.m.functions:
        for blk in fn.blocks:
            insts = list(blk.instructions)
            new_insts = []
            changed = False
            for inst in insts:
                si = getattr(inst, "sync_info", None)
                waits = list(si.on_wait) if (si is not None and si.on_wait) else []
                if len(waits) > 1:
                    changed = True
                    for w in waits[:-1]:
                        ctr += 1
                        new_insts.append(
                            mybir.InstNoOp(
                                name=f"waitsplit-{ctr}",
                                engine=inst.engine,
                                ins=[],
                                outs=[],
                                sync_info=mybir.SyncInfo(on_wait=[w], on_update=[]),
                            )
                        )
                    si.on_wait = [waits[-1]]
                new_insts.append(inst)
            if changed:
                blk.instructions = new_insts


def _patch_tile_drain(tile_mod, bass_mod):
    # Same walrus limitation hits the Tile kernel-tail drain (one wait per
    # ticked proc).  Chain the waits through single-wait sync NoOps.
    from concourse.vector_clock import ScopedClock, VectorClock

    def _drain_and_barrier(self, tick_clock, wait_clock):
        gc = tick_clock.global_clock
        n = len(gc)
        ticks = [gc[i] for i in range(n)]
        for p in [i for i in range(n) if ticks[i] > 0]:
            nop = self.nc.sync.nop(nofuse=True, hint="drain_wait_split")
            vc = VectorClock([ticks[j] if j == p else 0 for j in range(n)])
            wait_clock.add_sem_waits(nop.ins, ScopedClock({None: vc}))
        self.nc.sync.drain()
        self.nc.all_engine_barrier()
        assert self.sems is not None
        popped = self.nc._tile_sem_poison_stack.pop()
        assert popped is self._sem_poison
        self.nc.clear_and_free_semaphores(list(self.sems.allocated().values()))
        self.nc.all_engine_barrier()

    tile_mod.TileContext._drain_and_barrier = _drain_and_barrier


def _build():
    if "nc" in _cache:
        return _cache["nc"]

    import concourse.bass as bass
    import concourse.mybir as mybir
    import concourse.tile as tile

    _patch_tile_drain(tile, bass)

    f32 = mybir.dt.float32
    f32r = mybir.dt.float32r
    Exp = mybir.ActivationFunctionType.Exp

    nc = bass.Bass()
    xT = nc.dram_tensor("xT", [DM, S], f32r, kind="ExternalInput")
    Wq = nc.dram_tensor("Wq", [DM, 512], f32r, kind="ExternalInput")
    Wk = nc.dram_tensor("Wk", [DM, 512], f32r, kind="ExternalInput")
    Wv = nc.dram_tensor("Wv", [DM, 512], f32r, kind="ExternalInput")
    Wo = nc.dram_tensor("Wo", [512, DM], f32r, kind="ExternalInput")
    qkb = nc.dram_tensor("qkb", [128, 8], f32, kind="ExternalInput")
    vbb = nc.dram_tensor("vbb", [128, 512], f32, kind="ExternalInput")
    msk = nc.dram_tensor("msk", [128, 896], f32, kind="ExternalInput")
    onz = nc.dram_tensor("onz", [128, 65], f32r, kind="ExternalInput")
    y = nc.dram_tensor("y", [S, DM], f32, kind="ExternalOutput")
    dd1s = [nc.dram_tensor(f"dd1_{k}", [128, 8], f32, kind="Internal")
            for k in range(2)]
    dd2s = [nc.dram_tensor(f"dd2_{k}", [128, 8], f32r, kind="Internal")
            for k in range(2)]

    with tile.TileContext(nc) as tc:
        with nc.allow_low_precision(reason="fp32r tiles feeding the PE"), \
             tc.tile_pool(name="mp", bufs=1) as mp, \
             tc.tile_pool(name="pp", bufs=1, space="PSUM") as pp:
            from contextlib import ExitStack
            _pa_ctx = ExitStack()
            apl = _pa_ctx.enter_context(tc.tile_pool(name="pa", bufs=1))

            # ---- constants ----
            qkb_sb = mp.tile([128, 8], f32, tag="qkb")
            nc.scalar.dma_start(qkb_sb[:], qkb[:])
            vbb_sb = mp.tile([128, 512], f32, tag="vbb")
            nc.scalar.dma_start(vbb_sb[:], vbb[:])
            msk_sb = mp.tile([128, 896], f32, tag="msk")
            nc.scalar.dma_start(msk_sb[:], msk[:])
            ones_sb = mp.tile([128, 65], f32r, tag="ones")
            nc.scalar.dma_start(ones_sb[:], onz[:])
            ones_col = ones_sb[:, 64:65]
            ones_row = ones_sb[:, 0:64]

            # ---- phase A: projections ----
            xt = []
            wv = []
            for m in range(MT):
                w = apl.tile([128, 512], f32r, tag=f"wv{m}", name=f"wv{m}")
                nc.scalar.dma_start(w[:], Wv[m * 128:(m + 1) * 128, :])
                wv.append(w)
                t = apl.tile([128, S], f32r, tag=f"xt{m}", name=f"xt{m}")
                nc.sync.dma_start(t[:], xT[m * 128:(m + 1) * 128, :])
                xt.append(t)

            # v: [p, h*65+d] per 128-row p-tile; col 65h+64 = ones so the
            # merged S@V matmul (M=65) also produces the softmax denominator
            v_sb = []
            for p in range(S // 128):
                ps = pp.tile([128, 512], f32, tag="proj", bufs=2)
                for m in range(MT):
                    nc.tensor.matmul(
                        ps[:], xt[m][:, p * 128:(p + 1) * 128], wv[m][:],
                        start=(m == 0), stop=(m == MT - 1))
                vt = mp.tile([128, 520], f32r, tag=f"v{p}")
                nc.vector.tensor_add(
                    vt.rearrange("p (h c) -> p h c", c=65)[:, :, 0:64],
                    ps.rearrange("p (h c) -> p h c", c=64),
                    vbb_sb.rearrange("p (h c) -> p h c", c=64))
                nc.vector.tensor_copy(
                    vt.rearrange("p (h c) -> p h c", c=65)[:, :, 64:65],
                    ones_sb[:, 0:8].rearrange("p (h c) -> p h c", c=1))
                v_sb.append(vt)

            # q,k: [hd(pair), seq] per pair
            qkT = {}
            for ti, (tname, W) in enumerate((("q", Wq), ("k", Wk))):
                for pri in range(NPAIR):
                    wtiles = []
                    for m in range(MT):
                        wt = apl.tile([128, 128], f32r, tag="wqk", bufs=17)
                        nc.gpsimd.dma_start(
                            wt[:], W[m * 128:(m + 1) * 128,
                                     pri * 128:(pri + 1) * 128])
                        wtiles.append(wt)
                    out = mp.tile([128, S], f32r, tag=f"{tname}T{pri}")
                    for pb in range(NQP):
                        ps = pp.tile([128, 512], f32, tag="proj", bufs=2)
                        for m in range(MT):
                            nc.tensor.matmul(
                                ps[:], wtiles[m][:],
                                xt[m][:, pb * 512:(pb + 1) * 512],
                                start=(m == 0), stop=(m == MT - 1))
                        nc.vector.tensor_scalar_add(
                            out[:, pb * 512:(pb + 1) * 512], ps[:],
                            qkb_sb[:, 4 * ti + pri:4 * ti + pri + 1])
                    qkT[(tname, pri)] = out

            # phase-A-only tiles (xT, W streams) free here; phase-B pool
            # reuses their SBUF space (stack order: mp below, pa/pb above).
            _pa_ctx.close()
            _pb_ctx = ExitStack()
            pbl = _pb_ctx.enter_context(tc.tile_pool(name="pb", bufs=1))

            # Wo tiles for phase C
            wo = []
            for pri in range(NPAIR):
                t = pbl.tile([128, DM], f32r, tag=f"wo{pri}")
                nc.sync.dma_start(t[:], Wo[pri * 128:(pri + 1) * 128, :])
                wo.append(t)

            # ---- phase B + C per qp block ----
            def emit_oproj(i, at_sb):
                for pt in range(4):
                    P = 4 * i + pt
                    for dm in range(2):
                        ps = pp.tile([128, 512], f32, tag="proj", bufs=2,
                                     name="ps_o")
                        for pri in range(NPAIR):
                            nc.tensor.matmul(
                                ps[:],
                                at_sb[pri][:, pt * 128:(pt + 1) * 128],
                                wo[pri][:, dm * 512:(dm + 1) * 512],
                                start=(pri == 0), stop=(pri == NPAIR - 1))
                        yt = pbl.tile([128, 512], f32, tag="y", bufs=3,
                                      name="yt")
                        if (2 * pt + dm) % 2 == 0:
                            nc.vector.tensor_copy(yt[:], ps[:])
                        else:
                            nc.scalar.copy(yt[:], ps[:])
                        nc.gpsimd.dma_start(
                            y[P * 128:(P + 1) * 128, dm * 512:(dm + 1) * 512],
                            yt[:])

            pending = None
            for i in range(NQP):
                at_sb = []
                for pri in range(NPAIR):
                    qT = qkT[("q", pri)]
                    kT = qkT[("k", pri)]
                    kmax = 4 * (i + 1)
                    # ad: head A -> cols 0:512, head B -> cols 512:1024;
                    # row 64 = softmax denominators (ones column of v)
                    ad = pp.tile([65, 1024], f32, tag="ad", bufs=1)
                    for j in range(kmax):
                        ev = pp.tile([128, 1024], f32, tag="ev", bufs=2)
                        # logitsT for both heads (row-tiled K=64 pair)
                        nc.tensor.matmul(
                            ev[:, 0:512],
                            kT[0:64, j * 128:(j + 1) * 128],
                            qT[0:64, i * 512:(i + 1) * 512],
                            start=True, stop=True)
                        nc.tensor.matmul(
                            ev[:, 512:1024],
                            kT[64:128, j * 128:(j + 1) * 128],
                            qT[64:128, i * 512:(i + 1) * 512],
                            start=True, stop=True)
                        sc = pbl.tile([128, 1024], f32r, tag="sc", bufs=8)
                        nc.scalar.activation(sc[:], ev[:], Exp, scale=0.125)
                        if j >= 4 * i:
                            o = (j - 4 * i) * 128
                            for h in range(2):
                                base = h * 512
                                if o > 0:
                                    nc.vector.memset(
                                        sc[:, base:base + o].bitcast(f32), 0.0)
                                nc.vector.tensor_mul(
                                    sc[:, base + o:base + o + 128],
                                    sc[:, base + o:base + o + 128],
                                    msk_sb[:, 384:512])
                        st = (j == 0)
                        sp = (j == kmax - 1)
                        vt = v_sb[j]
                        # merged S@V + denominator (M=65, lhsT = [v_h | 1])
                        for h in range(2):
                            lh = 2 * pri + h
                            nc.tensor.matmul(
                                ad[0:65, h * 512:h * 512 + 512],
                                vt[:, lh * 65:lh * 65 + 65],
                                sc[:, h * 512:h * 512 + 512],
                                start=st, stop=sp)
                    # 1/denom: spread the 1024 denominators over 128
                    # partitions by DMA so the iterative-divide reciprocal
                    # runs on 128 lanes (8 elems/lane) instead of one
                    adc = pbl.tile([65, 1024], f32, tag="adc", bufs=2)
                    nc.vector.tensor_copy(adc[:], ad[:])
                    dd1 = dd1s[(4 * i + pri) % 2][:, :]
                    nc.sync.dma_start(
                        dd1.rearrange("p c -> (p c)").rearrange(
                            "(o f) -> o f", o=1), adc[64:65, :])
                    dn = pbl.tile([128, 8], f32, tag="dn", bufs=2)
                    nc.sync.dma_start(dn[:], dd1)
                    dr = pbl.tile([128, 8], f32r, tag="dr", bufs=2)
                    nc.vector.reciprocal(dr[:], dn[:])
                    dd2 = dd2s[(4 * i + pri) % 2][:, :]
                    nc.sync.dma_start(dd2, dr[:])
                    bcs = pbl.tile([64, 1024], f32, tag="bcs", bufs=2)
                    nc.sync.dma_start(
                        bcs[:],
                        dd2.bitcast(f32).rearrange("p c -> (p c)").rearrange(
                            "(o f) -> o f", o=1).partition_broadcast(64))
                    at = pbl.tile([128, 512], f32r, tag="at", bufs=8)
                    nc.vector.tensor_mul(at[0:64, :], adc[0:64, 0:512],
                                         bcs[:, 0:512])
                    tmp = pbl.tile([64, 512], f32r, tag="tmp", bufs=2)
                    nc.vector.tensor_mul(tmp[:], adc[0:64, 512:1024],
                                         bcs[:, 512:1024])
                    # head B to partitions 64-127 (SBUF->SBUF DMA repack)
                    nc.sync.dma_start(at[64:128, :], tmp[:])
                    at_sb.append(at)
                    if pri == 0 and pending is not None:
                        emit_oproj(*pending)
                        pending = None
                pending = (i, at_sb)
            emit_oproj(*pending)
            _pb_ctx.close()

    _split_multi_waits(nc, mybir)
    _cache["nc"] = nc
    return nc


def _host_inputs(x, Q_w, Q_b, K_w, K_b, V_w, V_b, O_w):
    big = (np.arange(128)[:, None] <= np.arange(896)[None, :] - 384).astype(
        np.float32)
    in_maps = []
    for c in range(8):
        b, hs = c // 2, HPC * (c % 2)
        he = hs + HPC
        qb = Q_b[hs:he].reshape(512).astype(np.float32)
        kb = K_b[hs:he].reshape(512).astype(np.float32)
        qkb = np.zeros((128, 8), np.float32)
        for pri in range(NPAIR):
            qkb[:, pri] = qb[pri * 128:(pri + 1) * 128]
            qkb[:, 4 + pri] = kb[pri * 128:(pri + 1) * 128]
        in_maps.append({
            "xT": np.ascontiguousarray(x[b].T).astype(np.float32),
            "Wq": np.ascontiguousarray(
                Q_w[hs:he].transpose(1, 0, 2).reshape(DM, 512)).astype(np.float32),
            "Wk": np.ascontiguousarray(
                K_w[hs:he].transpose(1, 0, 2).reshape(DM, 512)).astype(np.float32),
            "Wv": np.ascontiguousarray(
                V_w[hs:he].transpose(1, 0, 2).reshape(DM, 512)).astype(np.float32),
            "Wo": np.ascontiguousarray(O_w[hs:he].reshape(512, DM)).astype(
                np.float32),
            "qkb": qkb,
            "vbb": np.tile(V_b[hs:he].reshape(1, 512), (128, 1)).astype(
                np.float32),
            "msk": big,
            "onz": np.ones((128, 65), np.float32),
        })
    return in_maps


def kernel(x, Q_w, Q_b, K_w, K_b, V_w, V_b, O_w, O_b, _trace=False):
    x = np.asarray(x, np.float32)
    args = [np.asarray(a, np.float32)
            for a in (Q_w, Q_b, K_w, K_b, V_w, V_b, O_w)]
    O_b = np.asarray(O_b, np.float32)

    nc = _build()
    from concourse.bass_utils import run_bass_kernel_spmd

    in_maps = _host_inputs(x, *args)
    res = run_bass_kernel_spmd(nc, in_maps, core_ids=list(range(8)),
                               trace=_trace)
    _cache["last_result"] = res
    out = np.empty((B, S, DM), np.float32)
    for b in range(B):
        out[b] = res.results[2 * b]["y"] + res.results[2 * b + 1]["y"] + O_b
    return out


if __name__ == "__main__":
    # quick self-run with random inputs
    rng = np.random.default_rng(0)
    x = rng.standard_normal((B, S, DM), dtype=np.float32)
    shp = dict(Q_w=(H, DM, DH), Q_b=(H, DH), K_w=(H, DM, DH), K_b=(H, DH),
               V_w=(H, DM, DH), V_b=(H, DH), O_w=(H, DH, DM), O_b=(DM,))
    ins = {k: rng.standard_normal(v, dtype=np.float32) * 0.05
           for k, v in shp.items()}
    out = kernel(x, **ins)
    print("ran", out.shape, out.dtype)



# revision 4
# speedup vs baseline: 1.3887x; 1.3887x over previous
"""Multi-head causal attention (B=4, S=2048, H=16, Dh=64, Dm=1024) on 8
Trainium2 NeuronCores.

Sharding: core c handles batch b = c//2 and heads [8*(c%2), 8*(c%2)+8).
Each core computes its 8 heads' full attention + O-projection partial sum;
the host adds the two half-head partials per batch plus O_b.

v2 (all matmul operands bf16, PSUM accumulation fp32):
  - software-pipelined: QK projection of pair p+1 (and the O projection,
    for the last pair) is interleaved into the attention inner loop of
    pair p, so the ACT-engine exp stream (the attention bottleneck, ~1.15
    us per 128x1024 tile at 1 elem/lane/cycle) hides under independent PE
    work and the PE stays dense enough to keep the HAM clock gate at 2.4
    GHz.
  - logits per head-PAIR as 2 concurrent row-tiled K=64 matmuls (head A
    in PE rows 0-63, head B in rows 64-127).
  - diagonal k-tiles are N-trimmed: logits/exp/S@V only touch columns
    q >= k-tile start; the 128-wide staircase block is masked by one bf16
    multiply per head (no memset needed - trimmed S@V never reads the
    dead columns).
  - merged S@V (M=65, lhsT = [v_h | 1]) accumulates the softmax
    denominator in PSUM row 64; reciprocal is spread over 128 lanes via a
    DRAM bounce, then broadcast back with a partition-broadcast DMA.
"""

import sys

sys.path.insert(0, "/opt/trn_rl_repo")

import numpy as np
import ml_dtypes

BF16 = ml_dtypes.bfloat16

B, S, DM, H, DH = 4, 2048, 1024, 16, 64
HPC = 8          # heads per core
NPAIR = HPC // 2
PB = 512         # qp block width
NQP = S // PB    # 4 qp blocks
MT = DM // 128   # 8 m-tiles

_cache = {}


def _split_multi_waits(nc, mybir):
    # This container's walrus rejects >1 sync wait per instruction
    # ("Too many sync wait commands").  Move extra waits onto same-engine
    # NoOps right before the instruction; per-engine program order makes
    # this equivalent.
    ctr = 0
    for fn in nc.m.functions:
        for blk in fn.blocks:
            insts = list(blk.instructions)
            new_insts = []
            changed = False
            for inst in insts:
                si = getattr(inst, "sync_info", None)
                waits = list(si.on_wait) if (si is not None and si.on_wait) else []
                if len(waits) > 1:
                    changed = True
                    for w in waits[:-1]:
                        ctr += 1
                        new_insts.append(
                            mybir.InstNoOp(
                                name=f"waitsplit-{ctr}",
                                engine=inst.engine,
                                ins=[],
                                outs=[],
                                sync_info=mybir.SyncInfo(on_wait=[w], on_update=[]),
                            )
                        )
                    si.on_wait = [waits[-1]]
                new_insts.append(inst)
            if changed:
                blk.instructions = new_insts


def _patch_tile_drain(tile_mod, bass_mod):
    # Same walrus limitation hits the Tile kernel-tail drain (one wait per
    # ticked proc).  Chain the waits through single-wait sync NoOps.
    from concourse.vector_clock import ScopedClock, VectorClock

    def _drain_and_barrier(self, tick_clock, wait_clock):
        gc = tick_clock.global_clock
        n = len(gc)
        ticks = [gc[i] for i in range(n)]
        for p in [i for i in range(n) if ticks[i] > 0]:
            nop = self.nc.sync.nop(nofuse=True, hint="drain_wait_split")
            vc = VectorClock([ticks[j] if j == p else 0 for j in range(n)])
            wait_clock.add_sem_waits(nop.ins, ScopedClock({None: vc}))
        self.nc.sync.drain()
        self.nc.all_engine_barrier()
        assert self.sems is not None
        popped = self.nc._tile_sem_poison_stack.pop()
        assert popped is self._sem_poison
        self.nc.clear_and_free_semaphores(list(self.sems.allocated().values()))
        self.nc.all_engine_barrier()

    tile_mod.TileContext._drain_and_barrier = _drain_and_barrier


def _build():
    if "nc" in _cache:
        return _cache["nc"]

    import concourse.bass as bass
    import concourse.mybir as mybir
    import concourse.tile as tile

    _patch_tile_drain(tile, bass)

    f32 = mybir.dt.float32
    bf16 = mybir.dt.bfloat16
    Exp = mybir.ActivationFunctionType.Exp

    nc = bass.Bass()
    xT = nc.dram_tensor("xT", [DM, S], bf16, kind="ExternalInput")
    Wq = nc.dram_tensor("Wq", [DM, 512], bf16, kind="ExternalInput")
    Wk = nc.dram_tensor("Wk", [DM, 512], bf16, kind="ExternalInput")
    Wv = nc.dram_tensor("Wv", [DM, 512], bf16, kind="ExternalInput")
    Wo = nc.dram_tensor("Wo", [512, DM], bf16, kind="ExternalInput")
    qkb = nc.dram_tensor("qkb", [128, 8], f32, kind="ExternalInput")
    vbb = nc.dram_tensor("vbb", [128, 512], f32, kind="ExternalInput")
    msk = nc.dram_tensor("msk", [128, 128], bf16, kind="ExternalInput")
    onz = nc.dram_tensor("onz", [128, 16], bf16, kind="ExternalInput")
    y = nc.dram_tensor("y", [S, DM], bf16, kind="ExternalOutput")
    dd1s = [nc.dram_tensor(f"dd1_{k}", [128, 8], bf16, kind="Internal")
            for k in range(4)]
    dd2s = [nc.dram_tensor(f"dd2_{k}", [128, 8], f32, kind="Internal")
            for k in range(4)]

    with tile.TileContext(nc) as tc:
        with nc.allow_low_precision(reason="bf16 operands feeding the PE"), \
             tc.tile_pool(name="mp", bufs=1) as mp, \
             tc.tile_pool(name="pp", bufs=1, space="PSUM") as pp:
            from contextlib import ExitStack

            # ---- constants ----
            qkb_sb = mp.tile([128, 8], f32, tag="qkb")
            nc.scalar.dma_start(qkb_sb[:], qkb[:])
            vbb_sb = mp.tile([128, 512], f32, tag="vbb")
            nc.scalar.dma_start(vbb_sb[:], vbb[:])
            msk_sb = mp.tile([128, 128], bf16, tag="msk")
            nc.scalar.dma_start(msk_sb[:], msk[:])
            ones_sb = mp.tile([128, 16], bf16, tag="ones")
            nc.scalar.dma_start(ones_sb[:], onz[:])

            # ---- input streams ----
            xt = []
            wv = []
            for m in range(MT):
                w = mp.tile([128, 512], bf16, tag=f"wv{m}")
                nc.scalar.dma_start(w[:], Wv[m * 128:(m + 1) * 128, :])
                wv.append(w)
                t = mp.tile([128, S], bf16, tag=f"xt{m}")
                nc.sync.dma_start(t[:], xT[m * 128:(m + 1) * 128, :])
                xt.append(t)
            wo = []
            for pri in range(NPAIR):
                t = mp.tile([128, DM], bf16, tag=f"wo{pri}")
                nc.sync.dma_start(t[:], Wo[pri * 128:(pri + 1) * 128, :])
                wo.append(t)

            def emit_wqk_dma(pri):
                tiles = {}
                for ti, W in ((0, Wq), (1, Wk)):
                    for m in range(MT):
                        wt = mp.tile([128, 128], bf16, tag="wqk", bufs=33,
                                     name=f"wqk{ti}_{pri}_{m}")
                        nc.gpsimd.dma_start(
                            wt[:], W[m * 128:(m + 1) * 128,
                                     pri * 128:(pri + 1) * 128])
                        tiles[(ti, m)] = wt
                return tiles

            wqk_next = emit_wqk_dma(0)

            # ---- phase V: value projection, m-major, 2 p-tiles/wave ----
            # (reuses the attention "ev" PSUM tag so the total stays at
            #  ev 2x2 + ad 2 + proj 2 = 8 banks)
            v_sb = [None] * (S // 128)
            for wave in range(8):
                evt = pp.tile([128, 1024], f32, tag="ev", bufs=2,
                              name=f"vps{wave}")
                for m in range(MT):
                    for u in range(2):
                        p = 2 * wave + u
                        nc.tensor.matmul(
                            evt[:, u * 512:(u + 1) * 512],
                            xt[m][:, p * 128:(p + 1) * 128], wv[m][:],
                            start=(m == 0), stop=(m == MT - 1))
                for u in range(2):
                    p = 2 * wave + u
                    vt = mp.tile([128, 520], bf16, tag=f"v{p}")
                    nc.vector.tensor_add(
                        vt.rearrange("p (h c) -> p h c", c=65)[:, :, 0:64],
                        evt[:, u * 512:(u + 1) * 512].rearrange(
                            "p (h c) -> p h c", c=64),
                        vbb_sb.rearrange("p (h c) -> p h c", c=64))
                    nc.vector.tensor_copy(
                        vt.rearrange("p (h c) -> p h c", c=65)[:, :, 64:65],
                        ones_sb[:, 0:8].rearrange("p (h c) -> p h c", c=1))
                    v_sb[p] = vt

            # ---- projection work chunks (interleave filler) ----
            qkT = {}

            def qk_chunks(pri, wtiles):
                # 8 chunks; each = one (pb, type) psum accumulation group
                for pb in range(NQP):
                    for ti, tname in ((0, "q"), (1, "k")):
                        def chunk(pb=pb, ti=ti, tname=tname, wtiles=wtiles,
                                  pri=pri):
                            if (tname, pri) not in qkT:
                                qkT[(tname, pri)] = mp.tile(
                                    [128, S], bf16, tag=f"{tname}T", bufs=2,
                                    name=f"{tname}T{pri}")
                            out = qkT[(tname, pri)]
                            psq = pp.tile([128, 512], f32, tag="proj", bufs=2)
                            for m in range(MT):
                                nc.tensor.matmul(
                                    psq[:], wtiles[(ti, m)][:],
                                    xt[m][:, pb * 512:(pb + 1) * 512],
                                    start=(m == 0), stop=(m == MT - 1))
                            nc.vector.tensor_scalar_add(
                                out[:, pb * 512:(pb + 1) * 512], psq[:],
                                qkb_sb[:, 4 * ti + pri:4 * ti + pri + 1])
                        yield chunk

            at_sb = {}

            def o_chunks(i):
                # 8 chunks; each = one (pt, dm) output tile of q-block i
                for pt in range(4):
                    for dm in range(2):
                        def chunk(pt=pt, dm=dm, i=i):
                            P = 4 * i + pt
                            pso = pp.tile([128, 512], f32, tag="proj", bufs=2,
                                          name="ps_o")
                            for pri in range(NPAIR):
                                nc.tensor.matmul(
                                    pso[:],
                                    at_sb[(i, pri)][:, pt * 128:(pt + 1) * 128],
                                    wo[pri][:, dm * 512:(dm + 1) * 512],
                                    start=(pri == 0), stop=(pri == NPAIR - 1))
                            yt = mp.tile([128, 512], bf16, tag="y", bufs=3,
                                         name="yt")
                            nc.vector.tensor_copy(yt[:], pso[:])
                            nc.gpsimd.dma_start(
                                y[P * 128:(P + 1) * 128,
                                  dm * 512:(dm + 1) * 512], yt[:])
                        yield chunk

            def denom_chain(i, pri, ad):
                slot = (4 * i + pri) % 4
                adc = mp.tile([65, 1024], bf16, tag="adc", bufs=2)
                nc.vector.tensor_copy(adc[:], ad[:])
                dd1 = dd1s[slot][:, :]
                nc.sync.dma_start(
                    dd1.rearrange("p c -> (p c)").rearrange(
                        "(o f) -> o f", o=1), adc[64:65, :])
                dn = mp.tile([128, 8], bf16, tag="dn", bufs=2)
                nc.sync.dma_start(dn[:], dd1)
                dr = mp.tile([128, 8], f32, tag="dr", bufs=2)
                nc.vector.reciprocal(dr[:], dn[:])
                dd2 = dd2s[slot][:, :]
                nc.sync.dma_start(dd2, dr[:])
                bcs = mp.tile([64, 1024], f32, tag="bcs", bufs=2)
                nc.sync.dma_start(
                    bcs[:],
                    dd2.rearrange("p c -> (p c)").rearrange(
                        "(o f) -> o f", o=1).partition_broadcast(64))
                at = mp.tile([128, 512], bf16, tag="at", bufs=17,
                             name=f"at{i}_{pri}")
                nc.vector.tensor_mul(at[0:64, :], adc[0:64, 0:512],
                                     bcs[:, 0:512])
                tmp = mp.tile([64, 512], bf16, tag="tmp", bufs=2)
                nc.vector.tensor_mul(tmp[:], adc[0:64, 512:1024],
                                     bcs[:, 512:1024])
                nc.sync.dma_start(at[64:128, :], tmp[:])
                at_sb[(i, pri)] = at

            # ---- attention, software-pipelined by one j-step ----
            pend_sv = None      # S@V of the previous j (waits on its exp)
            pend_denom = None   # denominator chain of the previous i-block

            for pri in range(NPAIR):
                if pri == 0:
                    for ch in qk_chunks(0, wqk_next):
                        ch()  # lead-in, not overlapped
                wtiles = wqk_next
                if pri < NPAIR - 1:
                    wqk_next = emit_wqk_dma(pri + 1)
                    fill = list(qk_chunks(pri + 1, wqk_next))
                    ftot = len(fill)
                else:
                    fill = []
                    ftot = 24  # O chunks for blocks 0..2 arrive during pair 3
                fidx = 0
                gctr = 0
                ngroups = sum(4 * (ii + 1) for ii in range(NQP))  # 40 j-steps
                qT = qkT[("q", pri)]
                kT = qkT[("k", pri)]
                for i in range(NQP):
                    if pri == NPAIR - 1 and i > 0:
                        fill = fill + list(o_chunks(i - 1))
                    kmax = 4 * (i + 1)
                    ad = pp.tile([65, 1024], f32, tag="ad", bufs=1)
                    for j in range(kmax):
                        o = (j - 4 * i) * 128 if j >= 4 * i else 0
                        ev = pp.tile([128, 1024], f32, tag="ev", bufs=2)
                        nc.tensor.matmul(
                            ev[:, o:512],
                            kT[0:64, j * 128:(j + 1) * 128],
                            qT[0:64, i * 512 + o:(i + 1) * 512],
                            start=True, stop=True)
                        nc.tensor.matmul(
                            ev[:, 512 + o:1024],
                            kT[64:128, j * 128:(j + 1) * 128],
                            qT[64:128, i * 512 + o:(i + 1) * 512],
                            start=True, stop=True)
                        sc = mp.tile([128, 1024], bf16, tag="sc", bufs=4)
                        nc.scalar.activation(sc[:, o:1024], ev[:, o:1024],
                                             Exp, scale=0.125)
                        if j >= 4 * i:
                            for h in range(2):
                                cb = h * 512 + o
                                nc.vector.tensor_mul(
                                    sc[:, cb:cb + 128],
                                    sc[:, cb:cb + 128], msk_sb[:, :])
                        if pend_sv is not None:
                            pend_sv()
                        if pend_denom is not None:
                            pend_denom()
                            pend_denom = None
                        if fidx < len(fill) and fidx * ngroups <= gctr * ftot:
                            fill[fidx]()
                            fidx += 1
                        gctr += 1

                        def mk_sv(j=j, o=o, sc=sc, ad=ad, kmax=kmax, pri=pri):
                            def run():
                                st = (j == 0)
                                sp = (j == kmax - 1)
                                vt = v_sb[j]
                                for h in range(2):
                                    lh = 2 * pri + h
                                    nc.tensor.matmul(
                                        ad[0:65, h * 512 + o:h * 512 + 512],
                                        vt[:, lh * 65:lh * 65 + 65],
                                        sc[:, h * 512 + o:h * 512 + 512],
                                        start=st, stop=sp)
                            return run
                        pend_sv = mk_sv()
                    # close the i-block: flush its last S@V, then queue the
                    # denominator chain to be emitted inside the next block
                    pend_sv()
                    pend_sv = None

                    def mk_denom(i=i, pri=pri, ad=ad):
                        def run():
                            denom_chain(i, pri, ad)
                        return run
                    pend_denom = mk_denom()
                # drain leftover fill chunks at pair end
                if pend_denom is not None:
                    pend_denom()
                    pend_denom = None
                while fidx < len(fill):
                    fill[fidx]()
                    fidx += 1
            for ch in o_chunks(NQP - 1):
                ch()

    _split_multi_waits(nc, mybir)
    _cache["nc"] = nc
    return nc


def _host_inputs(x, Q_w, Q_b, K_w, K_b, V_w, V_b, O_w):
    stair = (np.arange(128)[:, None] <= np.arange(128)[None, :]).astype(
        np.float32)
    in_maps = []
    for c in range(8):
        b, hs = c // 2, HPC * (c % 2)
        he = hs + HPC
        qb = Q_b[hs:he].reshape(512).astype(np.float32)
        kb = K_b[hs:he].reshape(512).astype(np.float32)
        qkbm = np.zeros((128, 8), np.float32)
        for pri in range(NPAIR):
            qkbm[:, pri] = qb[pri * 128:(pri + 1) * 128]
            qkbm[:, 4 + pri] = kb[pri * 128:(pri + 1) * 128]
        in_maps.append({
            "xT": np.ascontiguousarray(x[b].T).astype(BF16),
            "Wq": np.ascontiguousarray(
                Q_w[hs:he].transpose(1, 0, 2).reshape(DM, 512)).astype(BF16),
            "Wk": np.ascontiguousarray(
                K_w[hs:he].transpose(1, 0, 2).reshape(DM, 512)).astype(BF16),
            "Wv": np.ascontiguousarray(
                V_w[hs:he].transpose(1, 0, 2).reshape(DM, 512)).astype(BF16),
            "Wo": np.ascontiguousarray(O_w[hs:he].reshape(512, DM)).astype(
                BF16),
            "qkb": qkbm,
            "vbb": np.tile(V_b[hs:he].reshape(1, 512), (128, 1)).astype(
                np.float32),
            "msk": stair.astype(BF16),
            "onz": np.ones((128, 16), BF16),
        })
    return in_maps


def kernel(x, Q_w, Q_b, K_w, K_b, V_w, V_b, O_w, O_b, _trace=False):
    x = np.asarray(x, np.float32)
    args = [np.asarray(a, np.float32)
            for a in (Q_w, Q_b, K_w, K_b, V_w, V_b, O_w)]
    O_b = np.asarray(O_b, np.float32)

    nc = _build()
    from concourse.bass_utils import run_bass_kernel_spmd

    in_maps = _host_inputs(x, *args)
    res = run_bass_kernel_spmd(nc, in_maps, core_ids=list(range(8)),
                               trace=_trace)
    _cache["last_result"] = res
    out = np.empty((B, S, DM), np.float32)
    for b in range(B):
        out[b] = (res.results[2 * b]["y"].astype(np.float32)
                  + res.results[2 * b + 1]["y"].astype(np.float32) + O_b)
    return out


if __name__ == "__main__":
    rng = np.random.default_rng(0)
    x = rng.standard_normal((B, S, DM), dtype=np.float32)
    shp = dict(Q_w=(H, DM, DH), Q_b=(H, DH), K_w=(H, DM, DH), K_b=(H, DH),
               V_w=(H, DM, DH), V_b=(H, DH), O_w=(H, DH, DM), O_b=(DM,))
    ins = {k: rng.standard_normal(v, dtype=np.float32) * 0.05
           for k, v in shp.items()}
    out = kernel(x, **ins)
    print("ran", out.shape, out.dtype)
